# revision 1
# baseline (speedup 1.0000x reference)
"""Multi-head causal self-attention (B=2, S=2048, D=2048, H=16) on 8 trn2
NeuronCores.

Sharding: tensor-parallel over heads. Core c owns heads {2c, 2c+1}:
  - QKV projection for its 2 heads (contraction over the full d_model),
  - causal attention for its 2 heads,
  - partial output projection  O_c = A_c @ W_out[:, c*256:(c+1)*256].T
Host sums the 8 partial outputs (the "all-reduce after out_proj" of the
TP scheme, done on host since the full output is assembled there anyway).

All on-device compute is laid out "feature-major" (transposed) so no
transposes are ever needed:
  - x is shipped as xT [D, B*S]
  - Q^T, K^T per head as [Dh=128, S];  V token-major as [S, Dh] tiles
  - scores are built transposed: S^T[k, q] = (K Q^T)[k, q]
  - softmax without max-subtraction (scores are O(+-5)), with the
    normalizer computed by a ones-row matmul over partition (key) dim and
    applied via a rank-1 PE broadcast.
  - attention output lands as A^T [Dh, S]; out-proj consumes it directly.

Matmuls run as float32r (full PE rate at free-dim >= 256, fp32 storage).
"""

import math

import ml_dtypes
import numpy as np

import concourse.bass as bass
import concourse.tile as tile
from concourse import bacc, mybir
from concourse.bass_utils import run_bass_kernel_spmd

F32 = mybir.dt.float32
F32R = mybir.dt.float32r
BF16 = mybir.dt.bfloat16

N_CORES = 8


class Cfg:
    def __init__(self, B=2, S=2048, D=2048, n_heads=16):
        self.B = B
        self.S = S
        self.D = D
        self.n_heads = n_heads
        self.Dh = 128
        self.DHT = n_heads * self.Dh       # W_qkv section stride (q/k/v)
        self.HPC = n_heads // N_CORES      # heads per core (2)
        self.QC = 512                      # token chunk (matmul free dim)
        self.KT = D // 128                 # k-tiles over d_model
        self.NCH = S // self.QC            # token chunks per batch
        assert self.HPC == 2 and D % 128 == 0 and S % self.QC == 0


def build_kernel(cfg: Cfg):
    """Build the SPMD single-core program. Returns compiled nc."""
    B, S, D, QC, KT, NCH = cfg.B, cfg.S, cfg.D, cfg.QC, cfg.KT, cfg.NCH
    Dh = cfg.Dh
    NQT = QC // 128                      # 128-token subtiles per chunk
    inv_sqrt_dh = 1.0 / math.sqrt(Dh)

    nc = bacc.Bacc("TRN2", target_bir_lowering=False, debug=False,
                   num_devices=N_CORES)

    xT = nc.dram_tensor("xT", [D, B * S], F32R, kind="ExternalInput").ap()
    wqkvT = nc.dram_tensor("wqkvT", [D, 768], F32R, kind="ExternalInput").ap()
    woutT = nc.dram_tensor("woutT", [256, D], F32R, kind="ExternalInput").ap()
    masks = nc.dram_tensor("masks", [128, NQT * QC], BF16,
                           kind="ExternalInput").ap()
    ones_col = nc.dram_tensor("ones_col", [128, 1], BF16,
                              kind="ExternalInput").ap()
    ones_row = nc.dram_tensor("ones_row", [1, 128], F32R,
                              kind="ExternalInput").ap()
    outT = nc.dram_tensor("outT", [D, B * S], F32, kind="ExternalOutput").ap()


    with tile.TileContext(nc) as tc:
        with (
            tc.tile_pool(name="wpool", bufs=1) as wpool,
            tc.tile_pool(name="xpool", bufs=5) as xpool,
            tc.tile_pool(name="qkvpool", bufs=1) as qkvpool,
            tc.tile_pool(name="apool", bufs=1) as apool,
            tc.tile_pool(name="ppool", bufs=7) as ppool,
            tc.tile_pool(name="opool", bufs=3) as opool,
            tc.tile_pool(name="smallpool", bufs=2) as smallpool,
            tc.tile_pool(name="pspool", bufs=4, space="PSUM") as pspool,
            tc.tile_pool(name="attnps", bufs=2, space="PSUM") as attnps,
            tc.tile_pool(name="rps", bufs=2, space="PSUM") as rps,
        ):
            # ---- static weights / constants ----
            w_tiles = []
            for k in range(KT):
                t = wpool.tile([128, 768], F32R, tag=f"w{k}", name=f"w{k}")
                nc.sync.dma_start(t[:], wqkvT[k * 128:(k + 1) * 128, :])
                w_tiles.append(t)
            wo_tiles = []
            for hh in range(2):
                t = wpool.tile([128, D], F32R, tag=f"wo{hh}", name=f"wo{hh}")
                nc.sync.dma_start(t[:], woutT[hh * 128:(hh + 1) * 128, :])
                wo_tiles.append(t)
            mask_t = wpool.tile([128, NQT * QC], BF16, tag="mask", name="mask")
            nc.sync.dma_start(mask_t[:], masks[:])
            onec_t = wpool.tile([128, 1], BF16, tag="onec", name="onec")
            nc.sync.dma_start(onec_t[:], ones_col[:])
            oner_t = wpool.tile([1, 128], F32R, tag="oner", name="oner")
            nc.sync.dma_start(oner_t[:], ones_row[:])

            for b in range(B):
                # ---- persistent per-batch QKV / A tiles ----
                # comps: 0=Q_h0 1=K_h0 2=Q_h1 3=K_h1 (dh-major [128, S])
                qk_sb = [qkvpool.tile([128, S], F32R, tag=f"qk{c}", name=f"qk{c}")
                         for c in range(4)]
                # V token-major: tile per 128 tokens, [128, 256] (2 heads)
                v_sb = [qkvpool.tile([128, 256], BF16, tag=f"v{t}", name=f"v{t}")
                        for t in range(S // 128)]
                # A^T per head [128, S]
                a_sb = [apool.tile([128, S], F32R, tag=f"a{h}", name=f"a{h}")
                        for h in range(2)]

                # ======== Phase A: QKV projection for this batch ========
                HKT = KT // 4
                for j in range(NCH):
                    col0 = b * S + j * QC
                    # four batched DMAs per chunk: [128, HKT*QC] quarters
                    # with the d_model k-tiles laid out along the free dim
                    halves = []
                    for hh in range(4):
                        t = xpool.tile([128, HKT * QC], F32R, tag="xt",
                                       name="xt")
                        src = xT[hh * HKT * 128:(hh + 1) * HKT * 128,
                                 col0:col0 + QC]
                        nc.sync.dma_start(
                            t[:].rearrange("p (k c) -> p k c", k=HKT),
                            src.rearrange("(k p) c -> p k c", p=128))
                        halves.append(t)

                    def xt_sl(k, f0, f1):
                        t = halves[k // HKT]
                        kk = k % HKT
                        return t[:, kk * QC + f0: kk * QC + f1]

                    # Q^T / K^T for both heads (copies on ScalarE: idle in
                    # this phase, keeps DVE free)
                    for c in range(4):
                        ps = pspool.tile([128, QC], F32, tag="ps", name="ps")
                        for k in range(KT):
                            nc.tensor.matmul(
                                ps[:],
                                (w_tiles[k][:, c * 128:(c + 1) * 128]),
                                (xt_sl(k, 0, QC)),
                                start=(k == 0), stop=(k == KT - 1))
                        nc.scalar.copy(
                            qk_sb[c][:, j * QC:(j + 1) * QC], ps[:])
                    # V token-major (both heads side by side)
                    for sub in range(NQT):
                        ps = pspool.tile([128, 256], F32, tag="ps", name="ps")
                        for k in range(KT):
                            nc.tensor.matmul(
                                ps[:],
                                (xt_sl(k, sub * 128, (sub + 1) * 128)),
                                (w_tiles[k][:, 512:768]),
                                start=(k == 0), stop=(k == KT - 1))
                        nc.scalar.copy(v_sb[j * NQT + sub][:], ps[:])

                # ======== Phase B+C: attention + out-proj per chunk ======
                # Normalization is software-pipelined one block behind so
                # the (slow) reciprocal never sits on the PE's in-order
                # path: block k's rank-1 broadcast + final mul are emitted
                # after block k+1's matmuls.

                def emit_attn_block(j, h):
                    # attnV/r matmuls lag the scores by SKEW k-tiles so the
                    # exp -> mask chain latency stays off the PE's in-order
                    # path.
                    SKEW = 2
                    n_kt = (j + 1) * QC // 128
                    qT = qk_sb[2 * h]
                    kTl = qk_sb[2 * h + 1]
                    attn = attnps.tile([128, QC], F32, tag="attn",
                                       name="attn")
                    r = rps.tile([1, QC], F32, tag="r", name="r")
                    p_tiles = {}

                    def emit_scores(kt):
                        rel = kt * 128 - j * QC
                        # causal trim: queries below the diagonal block's
                        # start contribute nothing. fp32r needs N>=256 for
                        # full rate, bf16 consumers can trim all the way.
                        f_sc = min(max(rel, 0), QC - 256)
                        f_av = max(rel, 0)
                        s_ps = pspool.tile([128, QC], F32, tag="ps",
                                           name="ps")
                        nc.tensor.matmul(
                            s_ps[:, f_sc:],
                            kTl[:, kt * 128:(kt + 1) * 128],
                            qT[:, j * QC + f_sc:(j + 1) * QC],
                            start=True, stop=True)
                        p_sb = ppool.tile([128, QC], BF16, tag="p", name="p")
                        nc.scalar.activation(
                            p_sb[:, f_av:], s_ps[:, f_av:],
                            mybir.ActivationFunctionType.Exp,
                            scale=inv_sqrt_dh)
                        if rel >= 0:
                            # diagonal block: zero the k > q half
                            ridx = rel // 128
                            nc.vector.tensor_mul(
                                p_sb[:, f_av:], p_sb[:, f_av:],
                                mask_t[:, ridx * QC + f_av:(ridx + 1) * QC])
                        p_tiles[kt] = (p_sb, f_av)

                    def emit_av(kt):
                        p_sb, f_av = p_tiles.pop(kt)
                        nc.tensor.matmul(
                            attn[:, f_av:],
                            v_sb[kt][:, h * 128:(h + 1) * 128],
                            p_sb[:, f_av:],
                            start=(kt == 0), stop=(kt == n_kt - 1))
                        nc.tensor.matmul(
                            r[:, f_av:], onec_t[:], p_sb[:, f_av:],
                            start=(kt == 0), stop=(kt == n_kt - 1))

                    for kt in range(n_kt):
                        emit_scores(kt)
                        if kt >= SKEW:
                            emit_av(kt - SKEW)
                    for kt in range(max(0, n_kt - SKEW), n_kt):
                        emit_av(kt)
                    # launch the reciprocal now (DVE), consumed one block
                    # later by the rank-1 broadcast
                    recip = smallpool.tile([1, QC], F32, tag="recip",
                                           name="recip")
                    nc.vector.reciprocal_approx_fast(recip[:], r[:])
                    recip_r = smallpool.tile([1, QC], F32R, tag="recipr",
                                             name="recipr")
                    nc.vector.tensor_copy(recip_r[:], recip[:])
                    return (j, h, attn, recip_r)

                def emit_finalize(blk):
                    j, h, attn, recip_r = blk
                    rb_ps = pspool.tile([128, QC], F32, tag="ps", name="ps")
                    nc.tensor.matmul(rb_ps[:], oner_t[:], recip_r[:],
                                     start=True, stop=True)
                    rb_sb = ppool.tile([128, QC], F32R, tag="p", name="p")
                    nc.vector.tensor_copy(rb_sb[:], rb_ps[:])
                    nc.vector.tensor_mul(
                        a_sb[h][:, j * QC:(j + 1) * QC], attn[:], rb_sb[:])

                def emit_outproj(j):
                    # partial over this core's 256 head-features; psum
                    # drains alternate DVE/ACT so the PE never waits on a
                    # slot, and the output DMAs ride the idle GpSimd SWDGE.
                    col0 = b * S + j * QC
                    for m in range(D // 128):
                        ps = pspool.tile([128, QC], F32, tag="ps", name="ps")
                        for h in range(2):
                            nc.tensor.matmul(
                                ps[:],
                                wo_tiles[h][:, m * 128:(m + 1) * 128],
                                a_sb[h][:, j * QC:(j + 1) * QC],
                                start=(h == 0), stop=(h == 1))
                        o_sb = opool.tile([128, QC], F32, tag="o", name="o")
                        nc.vector.tensor_copy(o_sb[:], ps[:])
                        nc.sync.dma_start(
                            outT[m * 128:(m + 1) * 128, col0:col0 + QC],
                            o_sb[:])

                pending = None
                for j in range(NCH):
                    for h in range(2):
                        blk = emit_attn_block(j, h)
                        if pending is not None:
                            emit_finalize(pending)
                            if pending[1] == 1:
                                emit_outproj(pending[0])
                        pending = blk
                emit_finalize(pending)
                emit_outproj(pending[0])

    nc.compile()
    return nc


def make_inputs(cfg: Cfg, x, W_qkv, W_out):
    """Host-side sharding: returns in_maps (list of 8 dicts)."""
    B, S, D = cfg.B, cfg.S, cfg.D
    Dh, QC, NQT = cfg.Dh, cfg.QC, cfg.QC // 128
    xTa = np.ascontiguousarray(
        x.reshape(B * S, D).T.astype(np.float32))          # [D, B*S]

    masks = np.zeros((128, NQT * QC), dtype=ml_dtypes.bfloat16)
    for ridx in range(NQT):
        rel = ridx * 128
        p = np.arange(128)[:, None]
        f = np.arange(QC)[None, :]
        masks[:, ridx * QC:(ridx + 1) * QC] = (p + rel <= f)
    ones_col = np.ones((128, 1), dtype=ml_dtypes.bfloat16)
    ones_row = np.ones((1, 128), dtype=np.float32)

    in_maps = []
    DHT = cfg.DHT
    for c in range(N_CORES):
        h0 = cfg.HPC * c
        wq = np.empty((D, 768), dtype=np.float32)          # [D, cols]
        for i, h in enumerate((h0, h0 + 1)):
            wq[:, (2 * i) * 128:(2 * i) * 128 + 128] = \
                W_qkv[0 * DHT + h * Dh: 0 * DHT + h * Dh + Dh, :].T   # Q_h
            wq[:, (2 * i + 1) * 128:(2 * i + 1) * 128 + 128] = \
                W_qkv[1 * DHT + h * Dh: 1 * DHT + h * Dh + Dh, :].T   # K_h
            wq[:, 512 + i * 128: 512 + (i + 1) * 128] = \
                W_qkv[2 * DHT + h * Dh: 2 * DHT + h * Dh + Dh, :].T   # V_h
        wo = np.ascontiguousarray(
            W_out[:, h0 * Dh:(h0 + cfg.HPC) * Dh].T.astype(np.float32))
        in_maps.append({
            "xT": xTa,
            "wqkvT": np.ascontiguousarray(wq),
            "woutT": wo,
            "masks": masks,
            "ones_col": ones_col,
            "ones_row": ones_row,
        })
    return in_maps


_CACHED = {}


def kernel(x, W_qkv, W_out, mask=None, **_ignored):
    cfg = Cfg(B=x.shape[0], S=x.shape[1], D=x.shape[2],
              n_heads=W_qkv.shape[0] // 384)
    key = (cfg.B, cfg.S, cfg.D)
    if key not in _CACHED:
        _CACHED[key] = build_kernel(cfg)
    nc = _CACHED[key]
    in_maps = make_inputs(cfg, np.asarray(x), np.asarray(W_qkv),
                          np.asarray(W_out))
    res = run_bass_kernel_spmd(nc, in_maps, list(range(N_CORES)))
    acc = res.results[0]["outT"].astype(np.float32)
    for c in range(1, N_CORES):
        acc = acc + res.results[c]["outT"]
    out = acc.T.reshape(cfg.B, cfg.S, cfg.D)
    return np.ascontiguousarray(out)



# revision 2
# speedup vs baseline: 1.0072x; 1.0072x over previous
"""Multi-head causal self-attention (B=2, S=2048, D=2048, H=16) on 8 trn2
NeuronCores — v5.

Sharding: tensor-parallel over heads. Core c owns heads {2c, 2c+1}. Host
sums the 8 bf16 partial outputs in f32.

v5 (= v4 pool/weave structure, engines rebalanced):
  - dedicated PSUM pools (qkv 2 / scores+rb 2 / outproj+r 2 / attn 2):
    scores bank-reuse no longer waits on outproj casts.
  - finalizeB emitted BEFORE the next attention block, so the 2-slot attn
    pool's WAR points backwards in every queue (deadlock-free).
  - outproj woven between attention tiles like the QKV projection.
  - normalizer accumulation on DVE (GpSimd tensor ops measured ~940ns per
    [128,512] add AND their SBUF-port contention inflated every DVE op
    ~2x; v4's GpSimd broadcast also sat in the finalize gating chain
    behind the racc backlog -> 19us PE gaps). GpSimd is left fully idle.
  - rank-1 reciprocal broadcast back on the PE (fast, and its bank comes
    from the scores pool whose rotation is recycled by the prompt exp).
"""

import math

import ml_dtypes
import numpy as np

import concourse.bass as bass
import concourse.tile as tile
from concourse import bacc, mybir
from concourse.bass_utils import run_bass_kernel_spmd

F32 = mybir.dt.float32
BF16 = mybir.dt.bfloat16

N_CORES = 8


class Cfg:
    def __init__(self, B=2, S=2048, D=2048, n_heads=16):
        self.B = B
        self.S = S
        self.D = D
        self.n_heads = n_heads
        self.Dh = 128
        self.DHT = n_heads * self.Dh       # W_qkv section stride (q/k/v)
        self.HPC = n_heads // N_CORES      # heads per core (2)
        self.QC = 512                      # token chunk (matmul free dim)
        self.KT = D // 128                 # k-tiles over d_model
        self.NCH = S // self.QC            # token chunks per batch
        assert self.HPC == 2 and D % 128 == 0 and S % self.QC == 0


def build_kernel(cfg: Cfg):
    """Build the SPMD single-core program. Returns compiled nc."""
    B, S, D, QC, KT, NCH = cfg.B, cfg.S, cfg.D, cfg.QC, cfg.KT, cfg.NCH
    NQT = QC // 128                      # 128-token subtiles per chunk
    HKT = KT // 4                        # k-tiles per x quarter tile
    inv_sqrt_dh = 1.0 / math.sqrt(cfg.Dh)

    nc = bacc.Bacc("TRN2", target_bir_lowering=False, debug=False,
                   num_devices=N_CORES)

    xT = nc.dram_tensor("xT", [D, B * S], BF16, kind="ExternalInput").ap()
    wqkvT = nc.dram_tensor("wqkvT", [D, 768], BF16, kind="ExternalInput").ap()
    woutT = nc.dram_tensor("woutT", [256, D], BF16, kind="ExternalInput").ap()
    masks = nc.dram_tensor("masks", [128, 128], BF16,
                           kind="ExternalInput").ap()
    ones_col = nc.dram_tensor("ones_col", [128, 1], BF16,
                              kind="ExternalInput").ap()
    ones_row = nc.dram_tensor("ones_row", [1, 128], BF16,
                              kind="ExternalInput").ap()
    outT = nc.dram_tensor("outT", [D, B * S], BF16, kind="ExternalOutput").ap()

    with tile.TileContext(nc) as tc:
        with (
            tc.tile_pool(name="wpool", bufs=1) as wpool,
            tc.tile_pool(name="xpool", bufs=12) as xpool,
            tc.tile_pool(name="qkpool", bufs=20) as qkpool,
            tc.tile_pool(name="vpool", bufs=20) as vpool,
            tc.tile_pool(name="apool", bufs=8) as apool,
            tc.tile_pool(name="ppool", bufs=7) as ppool,
            tc.tile_pool(name="rbpool", bufs=2) as rbpool,
            tc.tile_pool(name="rapool", bufs=3) as rapool,
            tc.tile_pool(name="opool", bufs=4) as opool,
            tc.tile_pool(name="smallpool", bufs=3) as smallpool,
            tc.tile_pool(name="qkvps", bufs=2, space="PSUM") as qkvps,
            tc.tile_pool(name="scoreps", bufs=2, space="PSUM") as scoreps,
            tc.tile_pool(name="outps", bufs=2, space="PSUM") as outps,
            tc.tile_pool(name="attnps", bufs=2, space="PSUM") as attnps,
        ):
            # ---- PE warm-up: tiny matmuls on zeroed scratch keep the HAM
            # activity window busy through the startup DMA wait.
            scratch = wpool.tile([128, 128], BF16, tag="scr", name="scr")
            nc.vector.memset(scratch[:], 0)
            wu_ps = scoreps.tile([128, 128], F32, tag="ps", name="ps")
            for _ in range(100):
                nc.tensor.matmul(wu_ps[0:32, 0:64], scratch[:, 0:32],
                                 scratch[:, 64:128], start=True, stop=True)

            # ---- x chunk DMA: 16 k-tile pieces into 4 quarter tiles.
            x_tiles = {}                   # (b, j) -> [4 quarter tiles]

            def ensure_x(b, j, interleave_w=False):
                if (b, j) in x_tiles or b >= B:
                    return
                col0 = b * S + j * QC
                quarts = [xpool.tile([128, HKT * QC], BF16, tag="xt",
                                     name="xt") for _ in range(4)]
                for k in range(KT):
                    t = quarts[k // HKT]
                    kk = k % HKT
                    nc.sync.dma_start(
                        t[:, kk * QC:(kk + 1) * QC],
                        xT[k * 128:(k + 1) * 128, col0:col0 + QC])
                    if interleave_w:
                        wt = wpool.tile([128, 768], BF16, tag=f"w{k}",
                                        name=f"w{k}")
                        nc.sync.dma_start(wt[:],
                                          wqkvT[k * 128:(k + 1) * 128, :])
                        w_tiles[k] = wt
                x_tiles[(b, j)] = quarts

            w_tiles = [None] * KT
            ensure_x(0, 0, interleave_w=True)
            ensure_x(0, 1)

            mask_t = wpool.tile([128, 128], BF16, tag="mask", name="mask")
            nc.sync.dma_start(mask_t[:], masks[:])
            onec_t = wpool.tile([128, 1], BF16, tag="onec", name="onec")
            nc.sync.dma_start(onec_t[:], ones_col[:])
            oner_t = wpool.tile([1, 128], BF16, tag="oner", name="oner")
            nc.sync.dma_start(oner_t[:], ones_row[:])
            wo_tiles = []
            for hh in range(2):
                t = wpool.tile([128, D], BF16, tag=f"wo{hh}", name=f"wo{hh}")
                nc.sync.dma_start(t[:], woutT[hh * 128:(hh + 1) * 128, :])
                wo_tiles.append(t)

            # per-(batch,chunk) result tiles
            qk_sb = {}                     # (b, c, j) -> [128, QC]
            v_sb = {}                      # (b, kt128) -> [128, 256]
            a_sb = {}                      # (b, h, j) -> [128, QC]

            def qkv_gen(b, j):
                """QKV projection for chunk (b, j) in 4-MM units."""
                ensure_x(b, j)
                halves = x_tiles[(b, j)]

                def xt_sl(k, f0, f1):
                    t = halves[k // HKT]
                    kk = k % HKT
                    return t[:, kk * QC + f0: kk * QC + f1]

                for c in range(4):
                    ps = qkvps.tile([128, QC], F32, tag="qps", name="qps")
                    for k in range(KT):
                        nc.tensor.matmul(
                            ps[:],
                            (w_tiles[k][:, c * 128:(c + 1) * 128]),
                            (xt_sl(k, 0, QC)),
                            start=(k == 0), stop=(k == KT - 1))
                        if k % 4 == 3:
                            yield
                    t = qkpool.tile([128, QC], BF16, tag="qk", name="qk")
                    nc.scalar.copy(t[:], ps[:])
                    qk_sb[(b, c, j)] = t
                for sub in range(NQT):
                    ps = qkvps.tile([128, 256], F32, tag="qps", name="qps")
                    for k in range(KT):
                        nc.tensor.matmul(
                            ps[:],
                            (xt_sl(k, sub * 128, (sub + 1) * 128)),
                            (w_tiles[k][:, 512:768]),
                            start=(k == 0), stop=(k == KT - 1))
                        if k % 4 == 3:
                            yield
                    t = vpool.tile([128, 256], BF16, tag="v", name="v")
                    nc.scalar.copy(t[:], ps[:])
                    v_sb[(b, j * NQT + sub)] = t

            def outproj_gen(b, j):
                """Output projection for chunk (b, j), one m-tile per unit."""
                col0 = b * S + j * QC
                for m in range(D // 128):
                    ps = outps.tile([128, QC], F32, tag="ops", name="ops")
                    for h in range(2):
                        nc.tensor.matmul(
                            ps[:],
                            wo_tiles[h][:, m * 128:(m + 1) * 128],
                            a_sb[(b, h, j)][:, :],
                            start=(h == 0), stop=(h == 1))
                    o_sb = opool.tile([128, QC], BF16, tag="o", name="o")
                    nc.vector.tensor_copy(o_sb[:], ps[:])
                    nc.sync.dma_start(
                        outT[m * 128:(m + 1) * 128, col0:col0 + QC],
                        o_sb[:])
                    yield

            weave_q = []                   # list of [key, gen]

            def start_gen(key, g):
                weave_q.append([key, g])

            def weave(n):
                """Pull n units round-robin from the pending generators."""
                pulled = 0
                i = 0
                while pulled < n and weave_q:
                    key, g = weave_q[i % len(weave_q)]
                    try:
                        next(g)
                        pulled += 1
                        i += 1
                    except StopIteration:
                        weave_q.remove([key, g])
                if not weave_q:
                    return

            def drain(key):
                for ent in list(weave_q):
                    if ent[0] == key:
                        for _ in ent[1]:
                            pass
                        weave_q.remove(ent)

            def drain_all():
                for ent in list(weave_q):
                    for _ in ent[1]:
                        pass
                    weave_q.remove(ent)

            def emit_attn_block(b, j, h, wu):
                SKEW = 2
                n_kt = (j + 1) * QC // 128
                attn = attnps.tile([128, QC], F32, tag="attn", name="attn")
                racc = rapool.tile([128, QC], BF16, tag="racc", name="racc")
                p_tiles = {}
                p0_hold = [None]

                def emit_scores(kt):
                    rel = kt * 128 - j * QC
                    f = max(rel, 0)
                    kq = qk_sb[(b, 2 * h + 1, kt // NQT)]
                    qq = qk_sb[(b, 2 * h, j)]
                    s_ps = scoreps.tile([128, QC], F32, tag="ps", name="ps")
                    nc.tensor.matmul(
                        s_ps[:, f:],
                        kq[:, (kt % NQT) * 128:(kt % NQT + 1) * 128],
                        qq[:, f:],
                        start=True, stop=True)
                    p_sb = ppool.tile([128, QC], BF16, tag="p", name="p")
                    nc.scalar.activation(
                        p_sb[:, f:], s_ps[:, f:],
                        mybir.ActivationFunctionType.Exp,
                        scale=inv_sqrt_dh)
                    if rel >= 0:
                        fe = min(f + 128, QC)
                        nc.vector.tensor_mul(
                            p_sb[:, f:fe], p_sb[:, f:fe],
                            mask_t[:, 0:fe - f])
                    p_tiles[kt] = (p_sb, f)

                def emit_av(kt):
                    p_sb, f = p_tiles.pop(kt)
                    if kt == 0:
                        p0_hold[0] = (p_sb, f)
                    nc.tensor.matmul(
                        attn[:, f:],
                        v_sb[(b, kt)][:, h * 128:(h + 1) * 128],
                        p_sb[:, f:],
                        start=(kt == 0), stop=(kt == n_kt - 1))
                    # normalizer accumulation on DVE
                    if kt == 0:
                        if n_kt > 1 and j > 0:
                            pass       # folded into kt==1's 3-operand add
                        else:
                            nc.vector.tensor_copy(racc[:], p_sb[:])
                    elif kt == 1:
                        if j == 0:
                            # racc holds kt=0's copy; kt=1 starts at col 128
                            nc.vector.tensor_add(
                                racc[:, f:], racc[:, f:], p_sb[:, f:])
                        else:
                            p0, _f0 = p0_hold[0]
                            nc.vector.tensor_add(racc[:], p0[:], p_sb[:])
                    else:
                        nc.vector.tensor_add(
                            racc[:, f:], racc[:, f:], p_sb[:, f:])

                for kt in range(n_kt):
                    emit_scores(kt)
                    if kt >= SKEW:
                        emit_av(kt - SKEW)
                    weave(wu)
                for kt in range(max(0, n_kt - SKEW), n_kt):
                    emit_av(kt)
                return {"b": b, "j": j, "h": h, "attn": attn, "racc": racc,
                        "recip_b": None, "done": False}

            def emit_finalize_a(blk):
                # r = ones^T racc on PE, reciprocal on DVE, partition
                # broadcast on GpSimd (SBUF->SBUF; no PSUM bank, no PE).
                r_ps = outps.tile([1, QC], F32, tag="ops", name="ops")
                nc.tensor.matmul(r_ps[:], onec_t[:], blk["racc"][:],
                                 start=True, stop=True)
                recip = smallpool.tile([1, QC], F32, tag="recip",
                                       name="recip")
                nc.vector.reciprocal_approx_fast(recip[:], r_ps[:])
                recip_b = smallpool.tile([1, QC], BF16, tag="recipb",
                                         name="recipb")
                nc.vector.tensor_copy(recip_b[:], recip[:])
                blk["recip_b"] = recip_b

            def emit_finalize_b(blk):
                rb_ps = scoreps.tile([128, QC], F32, tag="ps", name="ps")
                nc.tensor.matmul(rb_ps[:], oner_t[:], blk["recip_b"][:],
                                 start=True, stop=True)
                rb_sb = rbpool.tile([128, QC], BF16, tag="rb", name="rb")
                nc.vector.tensor_copy(rb_sb[:], rb_ps[:])
                t = apool.tile([128, QC], BF16, tag="a", name="a")
                nc.vector.tensor_mul(t[:], blk["attn"][:], rb_sb[:])
                a_sb[(blk["b"], blk["h"], blk["j"])] = t
                blk["done"] = True
                if blk["h"] == 1:
                    start_gen(("op", blk["b"], blk["j"]),
                              outproj_gen(blk["b"], blk["j"]))

            # ---- schedule ----
            # prologue: QKV(0,0) standalone (nothing to weave into)
            g0 = qkv_gen(0, 0)
            for _ in g0:
                pass
            blocks = []
            for b in range(B):
                for j in range(NCH):
                    nb, njj = (b, j + 2) if j + 2 < NCH else \
                        (b + 1, j + 2 - NCH)
                    if nb < B:
                        ensure_x(nb, njj)
                    wb, wj = (b, j + 1) if j + 1 < NCH else (b + 1, 0)
                    if wb < B:
                        start_gen(("qkv", wb, wj), qkv_gen(wb, wj))
                    n_kt = (j + 1) * NQT
                    wu = max(1, (48 + 2 * n_kt - 1) // (2 * n_kt))
                    for h in range(2):
                        n = len(blocks)
                        # finalizeB two blocks back, BEFORE this block, so
                        # the attn-slot WAR points backwards in DVE order
                        if n >= 2 and not blocks[n - 2]["done"]:
                            emit_finalize_b(blocks[n - 2])
                        blocks.append(emit_attn_block(b, j, h, wu))
                        if len(blocks) >= 2 and \
                                blocks[-2]["recip_b"] is None:
                            emit_finalize_a(blocks[-2])
                    # QKV(j+1) must be complete before attention(j+1)
                    drain(("qkv", wb, wj))
            emit_finalize_a(blocks[-1])
            emit_finalize_b(blocks[-2])
            emit_finalize_b(blocks[-1])
            drain_all()

    nc.compile()
    return nc


def make_inputs(cfg: Cfg, x, W_qkv, W_out):
    """Host-side sharding: returns in_maps (list of 8 dicts)."""
    B, S, D = cfg.B, cfg.S, cfg.D
    Dh = cfg.Dh
    xTa = np.ascontiguousarray(
        x.reshape(B * S, D).T).astype(ml_dtypes.bfloat16)   # [D, B*S]

    p = np.arange(128)[:, None]
    t = np.arange(128)[None, :]
    mask128 = (p <= t).astype(ml_dtypes.bfloat16)           # [128, 128]
    ones_col = np.ones((128, 1), dtype=ml_dtypes.bfloat16)
    ones_row = np.ones((1, 128), dtype=ml_dtypes.bfloat16)

    in_maps = []
    DHT = cfg.DHT
    for c in range(N_CORES):
        h0 = cfg.HPC * c
        wq = np.empty((D, 768), dtype=np.float32)           # [D, cols]
        for i, h in enumerate((h0, h0 + 1)):
            wq[:, (2 * i) * 128:(2 * i) * 128 + 128] = \
                W_qkv[0 * DHT + h * Dh: 0 * DHT + h * Dh + Dh, :].T   # Q_h
            wq[:, (2 * i + 1) * 128:(2 * i + 1) * 128 + 128] = \
                W_qkv[1 * DHT + h * Dh: 1 * DHT + h * Dh + Dh, :].T   # K_h
            wq[:, 512 + i * 128: 512 + (i + 1) * 128] = \
                W_qkv[2 * DHT + h * Dh: 2 * DHT + h * Dh + Dh, :].T   # V_h
        wo = np.ascontiguousarray(
            W_out[:, h0 * Dh:(h0 + cfg.HPC) * Dh].T).astype(
                ml_dtypes.bfloat16)
        in_maps.append({
            "xT": xTa,
            "wqkvT": wq.astype(ml_dtypes.bfloat16),
            "woutT": wo,
            "masks": mask128,
            "ones_col": ones_col,
            "ones_row": ones_row,
        })
    return in_maps


_CACHED = {}


def kernel(x, W_qkv, W_out, mask=None, **_ignored):
    cfg = Cfg(B=x.shape[0], S=x.shape[1], D=x.shape[2],
              n_heads=W_qkv.shape[0] // 384)
    key = (cfg.B, cfg.S, cfg.D)
    if key not in _CACHED:
        _CACHED[key] = build_kernel(cfg)
    nc = _CACHED[key]
    in_maps = make_inputs(cfg, np.asarray(x), np.asarray(W_qkv),
                          np.asarray(W_out))
    res = run_bass_kernel_spmd(nc, in_maps, list(range(N_CORES)))
    acc = res.results[0]["outT"].astype(np.float32)
    for c in range(1, N_CORES):
        acc = acc + res.results[c]["outT"].astype(np.float32)
    out = acc.T.reshape(cfg.B, cfg.S, cfg.D)
    return np.ascontiguousarray(out)


# revision 3
# speedup vs baseline: 1.0101x; 1.0029x over previous
"""Multi-head causal self-attention (B=2, S=2048, D=2048, H=16) on 8 trn2
NeuronCores — v8.

Sharding: tensor-parallel over heads. Core c owns heads {2c, 2c+1}. Host
sums the 8 bf16 partial outputs in f32.

v5 (= v4 pool/weave structure, engines rebalanced):
  - dedicated PSUM pools (qkv 2 / scores+rb 2 / outproj+r 2 / attn 2):
    scores bank-reuse no longer waits on outproj casts.
  - finalizeB emitted BEFORE the next attention block, so the 2-slot attn
    pool's WAR points backwards in every queue (deadlock-free).
  - outproj woven between attention tiles like the QKV projection.
  - normalizer accumulation on DVE (GpSimd tensor ops measured ~940ns per
    [128,512] add AND their SBUF-port contention inflated every DVE op
    ~2x; v4's GpSimd broadcast also sat in the finalize gating chain
    behind the racc backlog -> 19us PE gaps). GpSimd is left fully idle.
  - rank-1 reciprocal broadcast back on the PE (fast, and its bank comes
    from the scores pool whose rotation is recycled by the prompt exp).
"""

import math

import ml_dtypes
import numpy as np

import concourse.bass as bass
import concourse.tile as tile
from concourse import bacc, mybir
from concourse.bass_utils import run_bass_kernel_spmd

F32 = mybir.dt.float32
BF16 = mybir.dt.bfloat16

N_CORES = 8


class Cfg:
    def __init__(self, B=2, S=2048, D=2048, n_heads=16):
        self.B = B
        self.S = S
        self.D = D
        self.n_heads = n_heads
        self.Dh = 128
        self.DHT = n_heads * self.Dh       # W_qkv section stride (q/k/v)
        self.HPC = n_heads // N_CORES      # heads per core (2)
        self.QC = 512                      # token chunk (matmul free dim)
        self.KT = D // 128                 # k-tiles over d_model
        self.NCH = S // self.QC            # token chunks per batch
        assert self.HPC == 2 and D % 128 == 0 and S % self.QC == 0


def build_kernel(cfg: Cfg):
    """Build the SPMD single-core program. Returns compiled nc."""
    B, S, D, QC, KT, NCH = cfg.B, cfg.S, cfg.D, cfg.QC, cfg.KT, cfg.NCH
    NQT = QC // 128                      # 128-token subtiles per chunk
    HKT = KT // 4                        # k-tiles per x quarter tile
    inv_sqrt_dh = 1.0 / math.sqrt(cfg.Dh)

    nc = bacc.Bacc("TRN2", target_bir_lowering=False, debug=False,
                   num_devices=N_CORES)

    xT = nc.dram_tensor("xT", [D, B * S], BF16, kind="ExternalInput").ap()
    wqkvT = nc.dram_tensor("wqkvT", [D, 768], BF16, kind="ExternalInput").ap()
    woutT = nc.dram_tensor("woutT", [256, D], BF16, kind="ExternalInput").ap()
    masks = nc.dram_tensor("masks", [128, 128], BF16,
                           kind="ExternalInput").ap()
    ones_col = nc.dram_tensor("ones_col", [128, 1], BF16,
                              kind="ExternalInput").ap()
    ones_row = nc.dram_tensor("ones_row", [1, 128], BF16,
                              kind="ExternalInput").ap()
    outT = nc.dram_tensor("outT", [D, B * S], BF16, kind="ExternalOutput").ap()

    with tile.TileContext(nc) as tc:
        with (
            tc.tile_pool(name="wpool", bufs=1) as wpool,
            tc.tile_pool(name="xpool", bufs=12) as xpool,
            tc.tile_pool(name="qkpool", bufs=20) as qkpool,
            tc.tile_pool(name="vpool", bufs=20) as vpool,
            tc.tile_pool(name="apool", bufs=8) as apool,
            tc.tile_pool(name="ppool", bufs=7) as ppool,
            tc.tile_pool(name="rbpool", bufs=2) as rbpool,
            tc.tile_pool(name="rapool", bufs=3) as rapool,
            tc.tile_pool(name="opool", bufs=4) as opool,
            tc.tile_pool(name="smallpool", bufs=3) as smallpool,
            tc.tile_pool(name="qkvps", bufs=2, space="PSUM") as qkvps,
            tc.tile_pool(name="scoreps", bufs=2, space="PSUM") as scoreps,
            tc.tile_pool(name="outps", bufs=2, space="PSUM") as outps,
            tc.tile_pool(name="attnps", bufs=2, space="PSUM") as attnps,
        ):
            # ---- PE warm-up: tiny matmuls on zeroed scratch keep the HAM
            # activity window busy through the startup DMA wait.
            scratch = wpool.tile([128, 128], BF16, tag="scr", name="scr")
            nc.vector.memset(scratch[:], 0)
            wu_ps = scoreps.tile([128, 128], F32, tag="ps", name="ps")
            for _ in range(100):
                nc.tensor.matmul(wu_ps[0:32, 0:64], scratch[:, 0:32],
                                 scratch[:, 64:128], start=True, stop=True)

            # ---- x chunk DMA: 16 k-tile pieces into 4 quarter tiles.
            x_tiles = {}                   # (b, j) -> [4 quarter tiles]

            def ensure_x(b, j, fine=False, interleave_w=False):
                if (b, j) in x_tiles or b >= B:
                    return
                col0 = b * S + j * QC
                quarts = [xpool.tile([128, HKT * QC], BF16, tag="xt",
                                     name="xt") for _ in range(4)]
                if fine:
                    for k in range(KT):
                        t = quarts[k // HKT]
                        kk = k % HKT
                        nc.sync.dma_start(
                            t[:, kk * QC:(kk + 1) * QC],
                            xT[k * 128:(k + 1) * 128, col0:col0 + QC])
                        if interleave_w:
                            wt = wpool.tile([128, 768], BF16, tag=f"w{k}",
                                            name=f"w{k}")
                            nc.sync.dma_start(
                                wt[:], wqkvT[k * 128:(k + 1) * 128, :])
                            w_tiles[k] = wt
                else:
                    for hh in range(4):
                        src = xT[hh * HKT * 128:(hh + 1) * HKT * 128,
                                 col0:col0 + QC]
                        nc.sync.dma_start(
                            quarts[hh][:].rearrange("p (k c) -> p k c",
                                                    k=HKT),
                            src.rearrange("(k p) c -> p k c", p=128))
                x_tiles[(b, j)] = quarts

            w_tiles = [None] * KT
            ensure_x(0, 0, fine=True, interleave_w=True)
            ensure_x(0, 1, fine=True)

            mask_t = wpool.tile([128, 128], BF16, tag="mask", name="mask")
            nc.sync.dma_start(mask_t[:], masks[:])
            onec_t = wpool.tile([128, 1], BF16, tag="onec", name="onec")
            nc.sync.dma_start(onec_t[:], ones_col[:])
            oner_t = wpool.tile([1, 128], BF16, tag="oner", name="oner")
            nc.sync.dma_start(oner_t[:], ones_row[:])
            wo_tiles = []
            for hh in range(2):
                t = wpool.tile([128, D], BF16, tag=f"wo{hh}", name=f"wo{hh}")
                nc.sync.dma_start(t[:], woutT[hh * 128:(hh + 1) * 128, :])
                wo_tiles.append(t)

            # per-(batch,chunk) result tiles
            qk_sb = {}                     # (b, c, j) -> [128, QC]
            v_sb = {}                      # (b, kt128) -> [128, 256]
            a_sb = {}                      # (b, h, j) -> [128, QC]

            def qkv_gen(b, j):
                """QKV projection for chunk (b, j) in 4-MM units."""
                ensure_x(b, j)
                halves = x_tiles[(b, j)]

                def xt_sl(k, f0, f1):
                    t = halves[k // HKT]
                    kk = k % HKT
                    return t[:, kk * QC + f0: kk * QC + f1]

                for c in range(4):
                    ps = qkvps.tile([128, QC], F32, tag="qps", name="qps")
                    for k in range(KT):
                        nc.tensor.matmul(
                            ps[:],
                            (w_tiles[k][:, c * 128:(c + 1) * 128]),
                            (xt_sl(k, 0, QC)),
                            start=(k == 0), stop=(k == KT - 1))
                        if k % 4 == 3:
                            yield
                    t = qkpool.tile([128, QC], BF16, tag="qk", name="qk")
                    nc.scalar.copy(t[:], ps[:])
                    qk_sb[(b, c, j)] = t
                for sub in range(NQT):
                    ps = qkvps.tile([128, 256], F32, tag="qps", name="qps")
                    for k in range(KT):
                        nc.tensor.matmul(
                            ps[:],
                            (xt_sl(k, sub * 128, (sub + 1) * 128)),
                            (w_tiles[k][:, 512:768]),
                            start=(k == 0), stop=(k == KT - 1))
                        if k % 4 == 3:
                            yield
                    t = vpool.tile([128, 256], BF16, tag="v", name="v")
                    nc.scalar.copy(t[:], ps[:])
                    v_sb[(b, j * NQT + sub)] = t

            def outproj_gen(b, j):
                """Output projection for chunk (b, j), one m-tile per unit."""
                col0 = b * S + j * QC
                for m in range(D // 128):
                    ps = outps.tile([128, QC], F32, tag="ops", name="ops")
                    for h in range(2):
                        nc.tensor.matmul(
                            ps[:],
                            wo_tiles[h][:, m * 128:(m + 1) * 128],
                            a_sb[(b, h, j)][:, :],
                            start=(h == 0), stop=(h == 1))
                    o_sb = opool.tile([128, QC], BF16, tag="o", name="o")
                    if m % 2 == 0:
                        nc.vector.tensor_copy(o_sb[:], ps[:])
                    else:
                        nc.scalar.copy(o_sb[:], ps[:])
                    nc.gpsimd.dma_start(
                        outT[m * 128:(m + 1) * 128, col0:col0 + QC],
                        o_sb[:])
                    yield

            weave_q = []                   # list of [key, gen]

            def start_gen(key, g):
                weave_q.append([key, g])

            def weave(n):
                """Pull n units round-robin from the pending generators."""
                pulled = 0
                i = 0
                while pulled < n and weave_q:
                    key, g = weave_q[i % len(weave_q)]
                    try:
                        next(g)
                        pulled += 1
                        i += 1
                    except StopIteration:
                        weave_q.remove([key, g])
                if not weave_q:
                    return

            def drain(key):
                for ent in list(weave_q):
                    if ent[0] == key:
                        for _ in ent[1]:
                            pass
                        weave_q.remove(ent)

            def drain_all():
                for ent in list(weave_q):
                    for _ in ent[1]:
                        pass
                    weave_q.remove(ent)

            def emit_attn_block(b, j, h, wu):
                SKEW = 2
                n_kt = (j + 1) * QC // 128
                attn = attnps.tile([128, QC], F32, tag="attn", name="attn")
                racc = rapool.tile([128, QC], BF16, tag="racc", name="racc")
                p_tiles = {}
                p0_hold = [None]

                def emit_scores(kt):
                    rel = kt * 128 - j * QC
                    f = max(rel, 0)
                    kq = qk_sb[(b, 2 * h + 1, kt // NQT)]
                    qq = qk_sb[(b, 2 * h, j)]
                    s_ps = scoreps.tile([128, QC], F32, tag="ps", name="ps")
                    nc.tensor.matmul(
                        s_ps[:, f:],
                        kq[:, (kt % NQT) * 128:(kt % NQT + 1) * 128],
                        qq[:, f:],
                        start=True, stop=True)
                    p_sb = ppool.tile([128, QC], BF16, tag="p", name="p")
                    nc.scalar.activation(
                        p_sb[:, f:], s_ps[:, f:],
                        mybir.ActivationFunctionType.Exp,
                        scale=inv_sqrt_dh)
                    if rel >= 0:
                        fe = min(f + 128, QC)
                        nc.vector.tensor_mul(
                            p_sb[:, f:fe], p_sb[:, f:fe],
                            mask_t[:, 0:fe - f])
                    p_tiles[kt] = (p_sb, f)

                def emit_av(kt):
                    p_sb, f = p_tiles.pop(kt)
                    if kt == 0:
                        p0_hold[0] = (p_sb, f)
                    nc.tensor.matmul(
                        attn[:, f:],
                        v_sb[(b, kt)][:, h * 128:(h + 1) * 128],
                        p_sb[:, f:],
                        start=(kt == 0), stop=(kt == n_kt - 1))
                    # normalizer accumulation on DVE
                    if kt == 0:
                        if n_kt > 1 and j > 0:
                            pass       # folded into kt==1's 3-operand add
                        else:
                            nc.vector.tensor_copy(racc[:], p_sb[:])
                    elif kt == 1:
                        if j == 0:
                            # racc holds kt=0's copy; kt=1 starts at col 128
                            nc.vector.tensor_add(
                                racc[:, f:], racc[:, f:], p_sb[:, f:])
                        else:
                            p0, _f0 = p0_hold[0]
                            nc.vector.tensor_add(racc[:], p0[:], p_sb[:])
                    else:
                        nc.vector.tensor_add(
                            racc[:, f:], racc[:, f:], p_sb[:, f:])

                for kt in range(n_kt):
                    rel = kt * 128 - j * QC
                    emit_scores(kt)
                    if kt >= SKEW:
                        emit_av(kt - SKEW)
                    weave(wu + 3 if rel >= 0 else wu)
                for kt in range(max(0, n_kt - SKEW), n_kt):
                    emit_av(kt)
                return {"b": b, "j": j, "h": h, "attn": attn, "racc": racc,
                        "recip_b": None, "done": False}

            def emit_finalize_a(blk):
                # r = ones^T racc on PE, reciprocal on DVE, partition
                # broadcast on GpSimd (SBUF->SBUF; no PSUM bank, no PE).
                r_ps = outps.tile([1, QC], F32, tag="ops", name="ops")
                nc.tensor.matmul(r_ps[:], onec_t[:], blk["racc"][:],
                                 start=True, stop=True)
                recip = smallpool.tile([1, QC], F32, tag="recip",
                                       name="recip")
                nc.vector.reciprocal_approx_fast(recip[:], r_ps[:])
                recip_b = smallpool.tile([1, QC], BF16, tag="recipb",
                                         name="recipb")
                nc.vector.tensor_copy(recip_b[:], recip[:])
                blk["recip_b"] = recip_b

            def emit_finalize_b(blk):
                rb_ps = scoreps.tile([128, QC], F32, tag="ps", name="ps")
                nc.tensor.matmul(rb_ps[:], oner_t[:], blk["recip_b"][:],
                                 start=True, stop=True)
                rb_sb = rbpool.tile([128, QC], BF16, tag="rb", name="rb")
                nc.vector.tensor_copy(rb_sb[:], rb_ps[:])
                t = apool.tile([128, QC], BF16, tag="a", name="a")
                nc.vector.tensor_mul(t[:], blk["attn"][:], rb_sb[:])
                a_sb[(blk["b"], blk["h"], blk["j"])] = t
                blk["done"] = True
                if blk["h"] == 1:
                    start_gen(("op", blk["b"], blk["j"]),
                              outproj_gen(blk["b"], blk["j"]))

            # ---- schedule ----
            # prologue: QKV(0,0) standalone (nothing to weave into)
            g0 = qkv_gen(0, 0)
            for _ in g0:
                pass
            blocks = []
            for b in range(B):
                for j in range(NCH):
                    nb, njj = (b, j + 2) if j + 2 < NCH else \
                        (b + 1, j + 2 - NCH)
                    if nb < B:
                        ensure_x(nb, njj)
                    wb, wj = (b, j + 1) if j + 1 < NCH else (b + 1, 0)
                    if wb < B:
                        start_gen(("qkv", wb, wj), qkv_gen(wb, wj))
                    n_kt = (j + 1) * NQT
                    wu = max(1, (48 + 2 * n_kt - 1) // (2 * n_kt))
                    for h in range(2):
                        n = len(blocks)
                        # finalizeB two blocks back, BEFORE this block, so
                        # the attn-slot WAR points backwards in DVE order
                        if n >= 2 and not blocks[n - 2]["done"]:
                            emit_finalize_b(blocks[n - 2])
                        blocks.append(emit_attn_block(b, j, h, wu))
                        if len(blocks) >= 2 and \
                                blocks[-2]["recip_b"] is None:
                            emit_finalize_a(blocks[-2])
                    # QKV(j+1) must be complete before attention(j+1)
                    drain(("qkv", wb, wj))
            emit_finalize_a(blocks[-1])
            emit_finalize_b(blocks[-2])
            emit_finalize_b(blocks[-1])
            drain_all()

    nc.compile()
    return nc


def make_inputs(cfg: Cfg, x, W_qkv, W_out):
    """Host-side sharding: returns in_maps (list of 8 dicts)."""
    B, S, D = cfg.B, cfg.S, cfg.D
    Dh = cfg.Dh
    xTa = np.ascontiguousarray(
        x.reshape(B * S, D).T).astype(ml_dtypes.bfloat16)   # [D, B*S]

    p = np.arange(128)[:, None]
    t = np.arange(128)[None, :]
    mask128 = (p <= t).astype(ml_dtypes.bfloat16)           # [128, 128]
    ones_col = np.ones((128, 1), dtype=ml_dtypes.bfloat16)
    ones_row = np.ones((1, 128), dtype=ml_dtypes.bfloat16)

    in_maps = []
    DHT = cfg.DHT
    for c in range(N_CORES):
        h0 = cfg.HPC * c
        wq = np.empty((D, 768), dtype=np.float32)           # [D, cols]
        for i, h in enumerate((h0, h0 + 1)):
            wq[:, (2 * i) * 128:(2 * i) * 128 + 128] = \
                W_qkv[0 * DHT + h * Dh: 0 * DHT + h * Dh + Dh, :].T   # Q_h
            wq[:, (2 * i + 1) * 128:(2 * i + 1) * 128 + 128] = \
                W_qkv[1 * DHT + h * Dh: 1 * DHT + h * Dh + Dh, :].T   # K_h
            wq[:, 512 + i * 128: 512 + (i + 1) * 128] = \
                W_qkv[2 * DHT + h * Dh: 2 * DHT + h * Dh + Dh, :].T   # V_h
        wo = np.ascontiguousarray(
            W_out[:, h0 * Dh:(h0 + cfg.HPC) * Dh].T).astype(
                ml_dtypes.bfloat16)
        in_maps.append({
            "xT": xTa,
            "wqkvT": wq.astype(ml_dtypes.bfloat16),
            "woutT": wo,
            "masks": mask128,
            "ones_col": ones_col,
            "ones_row": ones_row,
        })
    return in_maps


_CACHED = {}


def kernel(x, W_qkv, W_out, mask=None, **_ignored):
    cfg = Cfg(B=x.shape[0], S=x.shape[1], D=x.shape[2],
              n_heads=W_qkv.shape[0] // 384)
    key = (cfg.B, cfg.S, cfg.D)
    if key not in _CACHED:
        _CACHED[key] = build_kernel(cfg)
    nc = _CACHED[key]
    in_maps = make_inputs(cfg, np.asarray(x), np.asarray(W_qkv),
                          np.asarray(W_out))
    res = run_bass_kernel_spmd(nc, in_maps, list(range(N_CORES)))
    acc = res.results[0]["outT"].astype(np.float32)
    for c in range(1, N_CORES):
        acc = acc + res.results[c]["outT"].astype(np.float32)
    out = acc.T.reshape(cfg.B, cfg.S, cfg.D)
    return np.ascontiguousarray(out)
